# revision 14
# baseline (speedup 1.0000x reference)
"""PagedAttention decode kernel for 8 Trainium2 NeuronCores.

Problem (hardcoded shapes):
  query        [32, 16, 128]  f32
  key_cache    [4096, 16, 16, 16, 8]   f32  ([NB, H, D/x, Bsz, x])
  value_cache  [4096, 16, 128, 16]     f32  ([NB, H, D, Bsz])
  block_tables [32, 128] i32
  context_lens [32] i32
  out          [32, 16, 128] f32

Sharding strategy: context-parallel. Every core processes all 32 sequences,
holding every-8th KV block of each sequence (~1/8 of each context). Each core
computes unnormalized attention numerators (sum_t exp(l_t) * v_t) and softmax
partial sums; the host sums partials across cores and normalizes. exp() is
taken without max-subtraction (logits are O(5), safe in fp32), so partial
softmax combination needs no max rescaling.

Device layout per core (host-prepared, bf16):
  kt [128, FK]  per seq: [d=128, h=16, L_s] K^T, zero-padded to L_s
  vt [128, FV]  per seq, per 128-token chunk: [t_chunk, h=16, d=128]
  qt [128, 512] q * 1/sqrt(128), laid out [d, s, h]
Outputs (f32):
  outp [128, 512]  unnormalized out, [d, s, h]
  acc  [16, 32]    sum over tokens of exp(logit) incl. zero-pad junk
                   (pad tokens give logit exactly 0 -> exp 1; host subtracts
                   the known pad count)

Pipeline per (core, seq), chunked by 128 tokens (nch = ceil(L/128) <= 2):
  1. DMA K^T tile [128, 16*L]
  2. QK matmuls per (head, chunk): lhsT = K_h chunk [d, tc] stationary,
     rhs = q_h [d, 1] -> logits^T [t, h] in PSUM (tokens on partitions)
  3. one ACT exp per seq: PSUM [128, nch*16] -> SBUF bf16 (junk rows beyond
     partial chunks are harmless, never read)
  4. denominator: ones-vector matmul  sum_t p[t, h] -> [1, 16] PSUM
  5. V matmuls per (chunk, head): out[d, h] += V_chunk_h^T p_chunk_h
  6. DVE copies PSUM->SBUF; one final DMA for all seqs
"""

import math

import numpy as np
import ml_dtypes

SCALE = 0.08838834764831845  # 1/sqrt(128)
NC = 8
S = 32
H = 16
D = 128
BSZ = 16
BF16 = ml_dtypes.bfloat16

# test.py hook: set _TRACE["on"]=True before calling kernel() to profile.
_TRACE = {"on": False, "result": None}


def _plan(cl):
    seqs = []
    fk = 0
    fv = 0
    for s in range(S):
        ctx = int(cl[s])
        nb = -(-ctx // BSZ)          # blocks in use for this seq
        pcb = -(-nb // NC)           # blocks per core (ceil)
        L = ((pcb * BSZ + 31) // 32) * 32   # per-core padded token count
        nch = -(-L // 128)           # 128-token chunks (last may be partial)
        seqs.append(dict(ctx=ctx, nb=nb, pcb=pcb, L=L, nch=nch, koff=fk, voff=fv))
        fk += H * L
        fv += nch * H * D
    return seqs, fk, fv


def _build(seqs, fk, fv):
    import concourse.mybir as mybir
    import concourse.tile as tile
    from concourse import bacc

    nc = bacc.Bacc("TRN2", target_bir_lowering=False, debug=False, num_devices=NC)
    kt = nc.dram_tensor("kt", [D, fk], mybir.dt.bfloat16, kind="ExternalInput").ap()
    vt = nc.dram_tensor("vt", [D, fv], mybir.dt.bfloat16, kind="ExternalInput").ap()
    qt = nc.dram_tensor("qt", [D, S * H], mybir.dt.bfloat16, kind="ExternalInput").ap()
    outp = nc.dram_tensor("outp", [D, S * H], mybir.dt.float32, kind="ExternalOutput").ap()
    acc = nc.dram_tensor("acc", [1, S * H], mybir.dt.float32, kind="ExternalOutput").ap()

    with tile.TileContext(nc) as tc:
        with tc.tile_pool(name="cst", bufs=1) as cst, \
             tc.tile_pool(name="kp", bufs=8) as kp, \
             tc.tile_pool(name="vp", bufs=8) as vp, \
             tc.tile_pool(name="pp", bufs=S) as pp, \
             tc.tile_pool(name="lg", bufs=6, space="PSUM") as lg, \
             tc.tile_pool(name="po", bufs=1, space="PSUM") as po, \
             tc.tile_pool(name="dn", bufs=1, space="PSUM") as dn:
            qsb = cst.tile([D, S * H], mybir.dt.bfloat16, tag="q")
            nc.sync.dma_start(qsb[:, :], qt[:, :])
            ones = cst.tile([D, 1], mybir.dt.bfloat16, tag="ones")
            nc.vector.memset(ones[:, :], 1.0)
            osb = cst.tile([D, S * H], mybir.dt.float32, tag="o")
            asb = cst.tile([1, S * H], mybir.dt.float32, tag="a")
            # all 32 sequences share one PSUM bank each for out/denominator:
            # no slot rotation -> PE never stalls on PSUM slot releases
            pout = po.tile([D, S * H], mybir.dt.float32, tag="po")
            dnp = dn.tile([1, S * H], mybir.dt.float32, tag="dn")

            for s, info in enumerate(seqs):
                L = info["L"]
                nch = info["nch"]
                koff = info["koff"]
                voff = info["voff"]

                ktile = kp.tile([D, H * L], mybir.dt.bfloat16, tag="k")
                nc.sync.dma_start(ktile[:, :], kt[:, koff:koff + H * L])

                vtile = vp.tile([D, nch * H * D], mybir.dt.bfloat16, tag="v")
                nfull = L // 128
                tcl = L - nfull * 128
                if nfull:
                    nc.sync.dma_start(
                        vtile[:, : nfull * H * D], vt[:, voff:voff + nfull * H * D]
                    )
                if tcl:
                    nc.sync.dma_start(
                        vtile[:tcl, nfull * H * D: nch * H * D],
                        vt[:tcl, voff + nfull * H * D: voff + nch * H * D],
                    )

                # logits^T: [token, head] per chunk, tokens on partitions
                ptp = lg.tile([D, nch * H], mybir.dt.float32, tag="lg")
                for c in range(nch):
                    tc_c = min(128, L - c * 128)
                    for h in range(H):
                        nc.tensor.matmul(
                            ptp[0:tc_c, c * H + h: c * H + h + 1],
                            ktile[:, h * L + c * 128: h * L + c * 128 + tc_c],
                            qsb[:, s * H + h:s * H + h + 1],
                            start=True,
                            stop=True,
                        )

                psb = pp.tile([D, nch * H], mybir.dt.bfloat16, tag="p")
                nc.scalar.activation(
                    psb[:, :], ptp[:, :], mybir.ActivationFunctionType.Exp
                )

                # one closed accumulation group per PSUM column: chunk loop
                # INSIDE the head loop (a start=True re-arms the whole 2KB
                # zero region, so open groups must not interleave per zone)
                for c in range(nch):
                    tc_c = min(128, L - c * 128)
                    nc.tensor.matmul(
                        dnp[0:1, s * H:(s + 1) * H],
                        ones[0:tc_c, :],
                        psb[0:tc_c, c * H:(c + 1) * H],
                        start=(c == 0),
                        stop=(c == nch - 1),
                    )
                for h in range(H):
                    for c in range(nch):
                        tc_c = min(128, L - c * 128)
                        nc.tensor.matmul(
                            pout[:, s * H + h:s * H + h + 1],
                            vtile[0:tc_c, c * H * D + h * D: c * H * D + (h + 1) * D],
                            psb[0:tc_c, c * H + h: c * H + h + 1],
                            start=(c == 0),
                            stop=(c == nch - 1),
                        )

            # ACT (not DVE) copies: keeps every matmul's cross-engine deps
            # on the single ACT semaphore — the MM ISA slot fits one wait.
            nc.scalar.copy(osb[:, :], pout[:, :])
            nc.scalar.copy(asb[:, :], dnp[:, :])
            nc.sync.dma_start(outp[:, :], osb[:, :])
            nc.sync.dma_start(acc[:, :], asb[:, :])
    nc.compile()
    return nc


def _shard_inputs(q, kc, vc, bt, cl, seqs, fk, fv):
    """Build per-core kt/vt and the shared qt."""
    kts = [np.zeros((D, fk), np.float32) for _ in range(NC)]
    vts = [np.zeros((D, fv), np.float32) for _ in range(NC)]

    for s, info in enumerate(seqs):
        ctx, nb, pcb, L = info["ctx"], info["nb"], info["pcb"], info["L"]
        koff, voff = info["koff"], info["voff"]
        idx = bt[s, :nb]
        kb = kc[idx]                       # [nb, H, 16, 16, 8] (copy)
        vb = vc[idx]                       # [nb, H, D, 16] (copy)
        rem = ctx - (nb - 1) * BSZ         # 1..16 tokens in last block
        if rem < BSZ:
            kb[-1, :, :, rem:, :] = 0
            vb[-1, :, :, rem:] = 0
        kbp = np.zeros((NC * pcb, H, BSZ, BSZ, 8), np.float32)
        kbp[:nb] = kb
        vbp = np.zeros((NC * pcb, H, D, BSZ), np.float32)
        vbp[:nb] = vb
        # core c owns blocks c, c+8, c+16, ... of this sequence
        kk = (
            kbp.reshape(pcb, NC, H, BSZ, BSZ, 8)
            .transpose(1, 3, 5, 2, 0, 4)
            .reshape(NC, D, H, pcb * BSZ)
        )
        vv = (
            vbp.reshape(pcb, NC, H, D, BSZ)
            .transpose(1, 0, 4, 2, 3)
            .reshape(NC, pcb * BSZ, H, D)
        )
        L0 = pcb * BSZ
        for c in range(NC):
            kpad = np.zeros((D, H, L), np.float32)
            kpad[:, :, :L0] = kk[c]
            kts[c][:, koff:koff + H * L] = kpad.reshape(D, H * L)
            vpad = np.zeros((L, H, D), np.float32)
            vpad[:L0] = vv[c]
            for ck in range(info["nch"]):
                tc_c = min(128, L - ck * 128)
                slab = vpad[ck * 128: ck * 128 + tc_c].reshape(tc_c, H * D)
                vts[c][0:tc_c, voff + ck * H * D: voff + (ck + 1) * H * D] = slab

    qtb = (
        (q.astype(np.float32) * SCALE)
        .transpose(2, 0, 1)
        .reshape(D, S * H)
        .astype(BF16)
    )
    ktb = [a.astype(BF16) for a in kts]
    vtb = [a.astype(BF16) for a in vts]
    return ktb, vtb, qtb


def kernel(query, key_cache, value_cache, block_tables, context_lens):
    from concourse import bass_utils

    q = np.asarray(query, dtype=np.float32)
    kc = np.asarray(key_cache, dtype=np.float32)
    vc = np.asarray(value_cache, dtype=np.float32)
    bt = np.asarray(block_tables).astype(np.int64)
    cl = np.asarray(context_lens).astype(np.int64)
    assert q.shape == (S, H, D) and bt.shape[0] == S and cl.shape == (S,)

    seqs, fk, fv = _plan(cl)
    nc = _build(seqs, fk, fv)
    ktb, vtb, qtb = _shard_inputs(q, kc, vc, bt, cl, seqs, fk, fv)

    in_maps = [{"kt": ktb[c], "vt": vtb[c], "qt": qtb} for c in range(NC)]
    kwargs = {}
    if _TRACE["on"]:
        kwargs = dict(trace=True, trace_cores=list(range(NC)))
    res = bass_utils.run_bass_kernel_spmd(
        nc, in_maps, core_ids=list(range(NC)), **kwargs
    )
    _TRACE["result"] = res

    o = np.zeros((D, S * H), np.float64)
    a = np.zeros((1, S * H), np.float64)
    for c in range(NC):
        o += res.results[c]["outp"].astype(np.float64)
        a += res.results[c]["acc"].astype(np.float64)
    a = a.reshape(S, H)
    for s, info in enumerate(seqs):
        a[s] -= NC * info["L"] - info["ctx"]  # remove exp(0)=1 pad junk
    out = o.reshape(D, S, H).transpose(1, 2, 0) / a[:, :, None]
    return out.astype(np.float32)


# revision 19
# speedup vs baseline: 1.0075x; 1.0075x over previous
"""PagedAttention decode kernel for 8 Trainium2 NeuronCores.

Problem (hardcoded shapes):
  query        [32, 16, 128]  f32
  key_cache    [4096, 16, 16, 16, 8]   f32  ([NB, H, D/x, Bsz, x])
  value_cache  [4096, 16, 128, 16]     f32  ([NB, H, D, Bsz])
  block_tables [32, 128] i32
  context_lens [32] i32
  out          [32, 16, 128] f32

Sharding strategy: context-parallel. Every core processes all 32 sequences,
holding every-8th KV block of each sequence (~1/8 of each context). Each core
computes unnormalized attention numerators (sum_t exp(l_t) * v_t) and softmax
partial sums; the host sums partials across cores and normalizes. exp() is
taken without max-subtraction (logits are O(5), safe in fp32), so partial
softmax combination needs no max rescaling.

Device layout per core (host-prepared, bf16):
  kt [128, FK]  per seq: [d=128, h=16, L_s] K^T, zero-padded to L_s
  vt [128, FV]  per seq, per 128-token chunk: [t_chunk, h=16, d=128]
  qt [128, 512] q * 1/sqrt(128), laid out [d, s, h]
Outputs (f32):
  outp [128, 512]  unnormalized out, [d, s, h]
  acc  [16, 32]    sum over tokens of exp(logit) incl. zero-pad junk
                   (pad tokens give logit exactly 0 -> exp 1; host subtracts
                   the known pad count)

Pipeline per (core, seq), chunked by 128 tokens (nch = ceil(L/128) <= 2):
  1. DMA K^T tile [128, 16*L]
  2. QK matmuls per (head, chunk): lhsT = K_h chunk [d, tc] stationary,
     rhs = q_h [d, 1] -> logits^T [t, h] in PSUM (tokens on partitions)
  3. one ACT exp per seq: PSUM [128, nch*16] -> SBUF bf16 (junk rows beyond
     partial chunks are harmless, never read)
  4. denominator: ones-vector matmul  sum_t p[t, h] -> [1, 16] PSUM
  5. V matmuls per (chunk, head): out[d, h] += V_chunk_h^T p_chunk_h
  6. DVE copies PSUM->SBUF; one final DMA for all seqs
"""

import math

import numpy as np
import ml_dtypes

SCALE = 0.08838834764831845  # 1/sqrt(128)
NC = 8
S = 32
H = 16
D = 128
BSZ = 16
BF16 = ml_dtypes.bfloat16

# test.py hook: set _TRACE["on"]=True before calling kernel() to profile.
_TRACE = {"on": False, "result": None}


def _plan(cl):
    seqs = []
    fk = 0
    fv = 0
    rot = 0
    for s in range(S):
        ctx = int(cl[s])
        nb = -(-ctx // BSZ)          # blocks in use for this seq
        pcb = -(-nb // NC)           # blocks per core (ceil)
        L = pcb * BSZ                # per-core token count (16-aligned)
        nch = -(-L // 128)           # 128-token chunks (last may be partial)
        # partial chunks occupy few partitions -> few DMA engines; rotate
        # their partition base (PE tile_position allows 0/32/64/96 only for
        # small tiles) to spread descriptor load
        tcl = L - (L // 128) * 128
        if 0 < tcl <= 32:
            bases = (0, 32, 64)
        elif 0 < tcl <= 64:
            bases = (0, 64)
        else:
            bases = (0,)
        pbase = bases[rot % len(bases)]
        rot += 1
        seqs.append(dict(ctx=ctx, nb=nb, pcb=pcb, L=L, nch=nch, koff=fk, voff=fv,
                         pbase=pbase))
        fk += H * L
        fv += nch * H * D
    return seqs, fk, fv


def _build(seqs, fk, fv):
    import concourse.mybir as mybir
    import concourse.tile as tile
    from concourse import bacc

    nc = bacc.Bacc("TRN2", target_bir_lowering=False, debug=False, num_devices=NC)
    kt = nc.dram_tensor("kt", [D, fk], mybir.dt.bfloat16, kind="ExternalInput").ap()
    vt = nc.dram_tensor("vt", [D, fv], mybir.dt.bfloat16, kind="ExternalInput").ap()
    qt = nc.dram_tensor("qt", [D, S * H], mybir.dt.bfloat16, kind="ExternalInput").ap()
    outp = nc.dram_tensor("outp", [D, S * H], mybir.dt.float32, kind="ExternalOutput").ap()
    acc = nc.dram_tensor("acc", [1, S * H], mybir.dt.float32, kind="ExternalOutput").ap()

    with tile.TileContext(nc) as tc:
        with tc.tile_pool(name="cst", bufs=1) as cst, \
             tc.tile_pool(name="kp", bufs=8) as kp, \
             tc.tile_pool(name="vp", bufs=8) as vp, \
             tc.tile_pool(name="pp", bufs=S) as pp, \
             tc.tile_pool(name="lg", bufs=6, space="PSUM") as lg, \
             tc.tile_pool(name="po", bufs=1, space="PSUM") as po, \
             tc.tile_pool(name="dn", bufs=1, space="PSUM") as dn:
            qsb = cst.tile([D, S * H], mybir.dt.bfloat16, tag="q")
            nc.sync.dma_start(qsb[:, :], qt[:, :])
            ones = cst.tile([D, 1], mybir.dt.bfloat16, tag="ones")
            nc.vector.memset(ones[:, :], 1.0)
            osb = cst.tile([D, S * H], mybir.dt.float32, tag="o")
            asb = cst.tile([1, S * H], mybir.dt.float32, tag="a")
            # all 32 sequences share one PSUM bank each for out/denominator:
            # no slot rotation -> PE never stalls on PSUM slot releases
            pout = po.tile([D, S * H], mybir.dt.float32, tag="po")
            dnp = dn.tile([1, S * H], mybir.dt.float32, tag="dn")

            for s, info in enumerate(seqs):
                L = info["L"]
                nch = info["nch"]
                koff = info["koff"]
                voff = info["voff"]

                ktile = kp.tile([D, H * L], mybir.dt.bfloat16, tag="k")
                nc.sync.dma_start(ktile[:, :], kt[:, koff:koff + H * L])

                vtile = vp.tile([D, nch * H * D], mybir.dt.bfloat16, tag="v")
                nfull = L // 128
                tcl = L - nfull * 128
                pb = info["pbase"]
                if nfull:
                    nc.sync.dma_start(
                        vtile[:, : nfull * H * D], vt[:, voff:voff + nfull * H * D]
                    )
                if tcl:
                    nc.sync.dma_start(
                        vtile[pb:pb + tcl, nfull * H * D: nch * H * D],
                        vt[pb:pb + tcl, voff + nfull * H * D: voff + nch * H * D],
                    )

                def cbase(c):
                    return pb if (tcl and c == nch - 1) else 0

                # logits^T: [token, head] per chunk, tokens on partitions
                ptp = lg.tile([D, nch * H], mybir.dt.float32, tag="lg")
                for c in range(nch):
                    tc_c = min(128, L - c * 128)
                    b = cbase(c)
                    for h in range(H):
                        nc.tensor.matmul(
                            ptp[b:b + tc_c, c * H + h: c * H + h + 1],
                            ktile[:, h * L + c * 128: h * L + c * 128 + tc_c],
                            qsb[:, s * H + h:s * H + h + 1],
                            start=True,
                            stop=True,
                        )

                psb = pp.tile([D, nch * H], mybir.dt.bfloat16, tag="p")
                nc.scalar.activation(
                    psb[:, :], ptp[:, :], mybir.ActivationFunctionType.Exp
                )

                # one closed accumulation group per PSUM column: chunk loop
                # INSIDE the head loop (a start=True re-arms the whole 2KB
                # zero region, so open groups must not interleave per zone)
                for c in range(nch):
                    tc_c = min(128, L - c * 128)
                    b = cbase(c)
                    nc.tensor.matmul(
                        dnp[0:1, s * H:(s + 1) * H],
                        ones[b:b + tc_c, :],
                        psb[b:b + tc_c, c * H:(c + 1) * H],
                        start=(c == 0),
                        stop=(c == nch - 1),
                    )
                for h in range(H):
                    for c in range(nch):
                        tc_c = min(128, L - c * 128)
                        b = cbase(c)
                        nc.tensor.matmul(
                            pout[:, s * H + h:s * H + h + 1],
                            vtile[b:b + tc_c, c * H * D + h * D: c * H * D + (h + 1) * D],
                            psb[b:b + tc_c, c * H + h: c * H + h + 1],
                            start=(c == 0),
                            stop=(c == nch - 1),
                        )

            # ACT (not DVE) copies: keeps every matmul's cross-engine deps
            # on the single ACT semaphore — the MM ISA slot fits one wait.
            nc.scalar.copy(osb[:, :], pout[:, :])
            nc.scalar.copy(asb[:, :], dnp[:, :])
            nc.sync.dma_start(outp[:, :], osb[:, :])
            nc.sync.dma_start(acc[:, :], asb[:, :])
    nc.compile()
    return nc


def _shard_inputs(q, kc, vc, bt, cl, seqs, fk, fv):
    """Build per-core kt/vt and the shared qt."""
    kts = [np.zeros((D, fk), np.float32) for _ in range(NC)]
    vts = [np.zeros((D, fv), np.float32) for _ in range(NC)]

    for s, info in enumerate(seqs):
        ctx, nb, pcb, L = info["ctx"], info["nb"], info["pcb"], info["L"]
        koff, voff = info["koff"], info["voff"]
        idx = bt[s, :nb]
        kb = kc[idx]                       # [nb, H, 16, 16, 8] (copy)
        vb = vc[idx]                       # [nb, H, D, 16] (copy)
        rem = ctx - (nb - 1) * BSZ         # 1..16 tokens in last block
        if rem < BSZ:
            kb[-1, :, :, rem:, :] = 0
            vb[-1, :, :, rem:] = 0
        kbp = np.zeros((NC * pcb, H, BSZ, BSZ, 8), np.float32)
        kbp[:nb] = kb
        vbp = np.zeros((NC * pcb, H, D, BSZ), np.float32)
        vbp[:nb] = vb
        # core c owns blocks c, c+8, c+16, ... of this sequence
        kk = (
            kbp.reshape(pcb, NC, H, BSZ, BSZ, 8)
            .transpose(1, 3, 5, 2, 0, 4)
            .reshape(NC, D, H, pcb * BSZ)
        )
        vv = (
            vbp.reshape(pcb, NC, H, D, BSZ)
            .transpose(1, 0, 4, 2, 3)
            .reshape(NC, pcb * BSZ, H, D)
        )
        L0 = pcb * BSZ
        for c in range(NC):
            kpad = np.zeros((D, H, L), np.float32)
            kpad[:, :, :L0] = kk[c]
            kts[c][:, koff:koff + H * L] = kpad.reshape(D, H * L)
            vpad = np.zeros((L, H, D), np.float32)
            vpad[:L0] = vv[c]
            nfull = L // 128
            for ck in range(info["nch"]):
                tc_c = min(128, L - ck * 128)
                b = info["pbase"] if (ck == nfull and tc_c < 128) else 0
                slab = vpad[ck * 128: ck * 128 + tc_c].reshape(tc_c, H * D)
                vts[c][b:b + tc_c, voff + ck * H * D: voff + (ck + 1) * H * D] = slab

    qtb = (
        (q.astype(np.float32) * SCALE)
        .transpose(2, 0, 1)
        .reshape(D, S * H)
        .astype(BF16)
    )
    ktb = [a.astype(BF16) for a in kts]
    vtb = [a.astype(BF16) for a in vts]
    return ktb, vtb, qtb


def kernel(query, key_cache, value_cache, block_tables, context_lens):
    from concourse import bass_utils

    q = np.asarray(query, dtype=np.float32)
    kc = np.asarray(key_cache, dtype=np.float32)
    vc = np.asarray(value_cache, dtype=np.float32)
    bt = np.asarray(block_tables).astype(np.int64)
    cl = np.asarray(context_lens).astype(np.int64)
    assert q.shape == (S, H, D) and bt.shape[0] == S and cl.shape == (S,)

    seqs, fk, fv = _plan(cl)
    nc = _build(seqs, fk, fv)
    ktb, vtb, qtb = _shard_inputs(q, kc, vc, bt, cl, seqs, fk, fv)

    in_maps = [{"kt": ktb[c], "vt": vtb[c], "qt": qtb} for c in range(NC)]
    kwargs = {}
    if _TRACE["on"]:
        kwargs = dict(trace=True, trace_cores=list(range(NC)))
    res = bass_utils.run_bass_kernel_spmd(
        nc, in_maps, core_ids=list(range(NC)), **kwargs
    )
    _TRACE["result"] = res

    o = np.zeros((D, S * H), np.float64)
    a = np.zeros((1, S * H), np.float64)
    for c in range(NC):
        o += res.results[c]["outp"].astype(np.float64)
        a += res.results[c]["acc"].astype(np.float64)
    a = a.reshape(S, H)
    for s, info in enumerate(seqs):
        a[s] -= NC * info["L"] - info["ctx"]  # remove exp(0)=1 pad junk
    out = o.reshape(D, S, H).transpose(1, 2, 0) / a[:, :, None]
    return out.astype(np.float32)


# revision 20
# speedup vs baseline: 1.0084x; 1.0009x over previous
"""PagedAttention decode kernel for 8 Trainium2 NeuronCores.

Problem (hardcoded shapes):
  query        [32, 16, 128]  f32
  key_cache    [4096, 16, 16, 16, 8]   f32  ([NB, H, D/x, Bsz, x])
  value_cache  [4096, 16, 128, 16]     f32  ([NB, H, D, Bsz])
  block_tables [32, 128] i32
  context_lens [32] i32
  out          [32, 16, 128] f32

Sharding strategy: context-parallel. Every core processes all 32 sequences,
holding every-8th KV block of each sequence (~1/8 of each context). Each core
computes unnormalized attention numerators (sum_t exp(l_t) * v_t) and softmax
partial sums; the host sums partials across cores and normalizes. exp() is
taken without max-subtraction (logits are O(5), safe in fp32), so partial
softmax combination needs no max rescaling.

Device layout per core (host-prepared, bf16):
  kt [128, FK]  per seq: [d=128, h=16, L_s] K^T, zero-padded to L_s
  vt [128, FV]  per seq, per 128-token chunk: [t_chunk, h=16, d=128]
  qt [128, 512] q * 1/sqrt(128), laid out [d, s, h]
Outputs (f32):
  outp [128, 512]  unnormalized out, [d, s, h]
  acc  [16, 32]    sum over tokens of exp(logit) incl. zero-pad junk
                   (pad tokens give logit exactly 0 -> exp 1; host subtracts
                   the known pad count)

Pipeline per (core, seq), chunked by 128 tokens (nch = ceil(L/128) <= 2):
  1. DMA K^T tile [128, 16*L]
  2. QK matmuls per (head, chunk): lhsT = K_h chunk [d, tc] stationary,
     rhs = q_h [d, 1] -> logits^T [t, h] in PSUM (tokens on partitions)
  3. one ACT exp per seq: PSUM [128, nch*16] -> SBUF bf16 (junk rows beyond
     partial chunks are harmless, never read)
  4. denominator: ones-vector matmul  sum_t p[t, h] -> [1, 16] PSUM
  5. V matmuls per (chunk, head): out[d, h] += V_chunk_h^T p_chunk_h
  6. DVE copies PSUM->SBUF; one final DMA for all seqs
"""

import math

import numpy as np
import ml_dtypes

SCALE = 0.08838834764831845  # 1/sqrt(128)
NC = 8
S = 32
H = 16
D = 128
BSZ = 16
BF16 = ml_dtypes.bfloat16

# test.py hook: set _TRACE["on"]=True before calling kernel() to profile.
_TRACE = {"on": False, "result": None}


def _plan(cl):
    seqs = []
    fk = 0
    fv = 0
    rot = 0
    for s in range(S):
        ctx = int(cl[s])
        nb = -(-ctx // BSZ)          # blocks in use for this seq
        pcb = -(-nb // NC)           # blocks per core (ceil)
        L = pcb * BSZ                # per-core token count (16-aligned)
        nch = -(-L // 128)           # 128-token chunks (last may be partial)
        # partial chunks occupy few partitions -> few DMA engines; rotate
        # their partition base (PE tile_position allows 0/32/64/96 only for
        # small tiles) to spread descriptor load
        tcl = L - (L // 128) * 128
        if 0 < tcl <= 32:
            bases = (0, 32, 64)
        elif 0 < tcl <= 64:
            bases = (0, 64)
        else:
            bases = (0,)
        pbase = bases[rot % len(bases)]
        rot += 1
        seqs.append(dict(ctx=ctx, nb=nb, pcb=pcb, L=L, nch=nch, koff=fk, voff=fv,
                         pbase=pbase))
        fk += H * L
        fv += nch * H * D
    return seqs, fk, fv


def _build(seqs, fk, fv):
    import concourse.mybir as mybir
    import concourse.tile as tile
    from concourse import bacc

    nc = bacc.Bacc("TRN2", target_bir_lowering=False, debug=False, num_devices=NC)
    kt = nc.dram_tensor("kt", [D, fk], mybir.dt.bfloat16, kind="ExternalInput").ap()
    vt = nc.dram_tensor("vt", [D, fv], mybir.dt.bfloat16, kind="ExternalInput").ap()
    qt = nc.dram_tensor("qt", [D, S * H], mybir.dt.bfloat16, kind="ExternalInput").ap()
    outp = nc.dram_tensor("outp", [D, S * H], mybir.dt.float32, kind="ExternalOutput").ap()
    acc = nc.dram_tensor("acc", [1, S * H], mybir.dt.float32, kind="ExternalOutput").ap()

    with tile.TileContext(nc) as tc:
        with tc.tile_pool(name="cst", bufs=1) as cst, \
             tc.tile_pool(name="kp", bufs=8) as kp, \
             tc.tile_pool(name="vp", bufs=8) as vp, \
             tc.tile_pool(name="pp", bufs=S) as pp, \
             tc.tile_pool(name="lg", bufs=6, space="PSUM") as lg, \
             tc.tile_pool(name="po", bufs=1, space="PSUM") as po, \
             tc.tile_pool(name="dn", bufs=1, space="PSUM") as dn:
            qsb = cst.tile([D, S * H], mybir.dt.bfloat16, tag="q")
            nc.sync.dma_start(qsb[:, :], qt[:, :])
            ones = cst.tile([D, 1], mybir.dt.bfloat16, tag="ones")
            nc.vector.memset(ones[:, :], 1.0)
            osb = cst.tile([D, S * H], mybir.dt.float32, tag="o")
            asb = cst.tile([1, S * H], mybir.dt.float32, tag="a")
            # all 32 sequences share one PSUM bank each for out/denominator:
            # no slot rotation -> PE never stalls on PSUM slot releases
            pout = po.tile([D, S * H], mybir.dt.float32, tag="po")
            dnp = dn.tile([1, S * H], mybir.dt.float32, tag="dn")

            for s, info in enumerate(seqs):
                L = info["L"]
                nch = info["nch"]
                koff = info["koff"]
                voff = info["voff"]

                ktile = kp.tile([D, H * L], mybir.dt.bfloat16, tag="k")
                nc.sync.dma_start(ktile[:, :], kt[:, koff:koff + H * L])

                vtile = vp.tile([D, nch * H * D], mybir.dt.bfloat16, tag="v")
                nfull = L // 128
                tcl = L - nfull * 128
                pb = info["pbase"]
                if nfull:
                    nc.scalar.dma_start(
                        vtile[:, : nfull * H * D], vt[:, voff:voff + nfull * H * D]
                    )
                if tcl:
                    nc.scalar.dma_start(
                        vtile[pb:pb + tcl, nfull * H * D: nch * H * D],
                        vt[pb:pb + tcl, voff + nfull * H * D: voff + nch * H * D],
                    )

                def cbase(c):
                    return pb if (tcl and c == nch - 1) else 0

                # logits^T: [token, head] per chunk, tokens on partitions
                ptp = lg.tile([D, nch * H], mybir.dt.float32, tag="lg")
                for c in range(nch):
                    tc_c = min(128, L - c * 128)
                    b = cbase(c)
                    for h in range(H):
                        nc.tensor.matmul(
                            ptp[b:b + tc_c, c * H + h: c * H + h + 1],
                            ktile[:, h * L + c * 128: h * L + c * 128 + tc_c],
                            qsb[:, s * H + h:s * H + h + 1],
                            start=True,
                            stop=True,
                        )

                psb = pp.tile([D, nch * H], mybir.dt.bfloat16, tag="p")
                nc.scalar.activation(
                    psb[:, :], ptp[:, :], mybir.ActivationFunctionType.Exp
                )

                # one closed accumulation group per PSUM column: chunk loop
                # INSIDE the head loop (a start=True re-arms the whole 2KB
                # zero region, so open groups must not interleave per zone)
                for c in range(nch):
                    tc_c = min(128, L - c * 128)
                    b = cbase(c)
                    nc.tensor.matmul(
                        dnp[0:1, s * H:(s + 1) * H],
                        ones[b:b + tc_c, :],
                        psb[b:b + tc_c, c * H:(c + 1) * H],
                        start=(c == 0),
                        stop=(c == nch - 1),
                    )
                for h in range(H):
                    for c in range(nch):
                        tc_c = min(128, L - c * 128)
                        b = cbase(c)
                        nc.tensor.matmul(
                            pout[:, s * H + h:s * H + h + 1],
                            vtile[b:b + tc_c, c * H * D + h * D: c * H * D + (h + 1) * D],
                            psb[b:b + tc_c, c * H + h: c * H + h + 1],
                            start=(c == 0),
                            stop=(c == nch - 1),
                        )

            # ACT (not DVE) copies: keeps every matmul's cross-engine deps
            # on the single ACT semaphore — the MM ISA slot fits one wait.
            nc.scalar.copy(osb[:, :], pout[:, :])
            nc.scalar.copy(asb[:, :], dnp[:, :])
            nc.sync.dma_start(outp[:, :], osb[:, :])
            nc.sync.dma_start(acc[:, :], asb[:, :])
    nc.compile()
    return nc


def _shard_inputs(q, kc, vc, bt, cl, seqs, fk, fv):
    """Build per-core kt/vt and the shared qt."""
    kts = [np.zeros((D, fk), np.float32) for _ in range(NC)]
    vts = [np.zeros((D, fv), np.float32) for _ in range(NC)]

    for s, info in enumerate(seqs):
        ctx, nb, pcb, L = info["ctx"], info["nb"], info["pcb"], info["L"]
        koff, voff = info["koff"], info["voff"]
        idx = bt[s, :nb]
        kb = kc[idx]                       # [nb, H, 16, 16, 8] (copy)
        vb = vc[idx]                       # [nb, H, D, 16] (copy)
        rem = ctx - (nb - 1) * BSZ         # 1..16 tokens in last block
        if rem < BSZ:
            kb[-1, :, :, rem:, :] = 0
            vb[-1, :, :, rem:] = 0
        kbp = np.zeros((NC * pcb, H, BSZ, BSZ, 8), np.float32)
        kbp[:nb] = kb
        vbp = np.zeros((NC * pcb, H, D, BSZ), np.float32)
        vbp[:nb] = vb
        # core c owns blocks c, c+8, c+16, ... of this sequence
        kk = (
            kbp.reshape(pcb, NC, H, BSZ, BSZ, 8)
            .transpose(1, 3, 5, 2, 0, 4)
            .reshape(NC, D, H, pcb * BSZ)
        )
        vv = (
            vbp.reshape(pcb, NC, H, D, BSZ)
            .transpose(1, 0, 4, 2, 3)
            .reshape(NC, pcb * BSZ, H, D)
        )
        L0 = pcb * BSZ
        for c in range(NC):
            kpad = np.zeros((D, H, L), np.float32)
            kpad[:, :, :L0] = kk[c]
            kts[c][:, koff:koff + H * L] = kpad.reshape(D, H * L)
            vpad = np.zeros((L, H, D), np.float32)
            vpad[:L0] = vv[c]
            nfull = L // 128
            for ck in range(info["nch"]):
                tc_c = min(128, L - ck * 128)
                b = info["pbase"] if (ck == nfull and tc_c < 128) else 0
                slab = vpad[ck * 128: ck * 128 + tc_c].reshape(tc_c, H * D)
                vts[c][b:b + tc_c, voff + ck * H * D: voff + (ck + 1) * H * D] = slab

    qtb = (
        (q.astype(np.float32) * SCALE)
        .transpose(2, 0, 1)
        .reshape(D, S * H)
        .astype(BF16)
    )
    ktb = [a.astype(BF16) for a in kts]
    vtb = [a.astype(BF16) for a in vts]
    return ktb, vtb, qtb


def kernel(query, key_cache, value_cache, block_tables, context_lens):
    from concourse import bass_utils

    q = np.asarray(query, dtype=np.float32)
    kc = np.asarray(key_cache, dtype=np.float32)
    vc = np.asarray(value_cache, dtype=np.float32)
    bt = np.asarray(block_tables).astype(np.int64)
    cl = np.asarray(context_lens).astype(np.int64)
    assert q.shape == (S, H, D) and bt.shape[0] == S and cl.shape == (S,)

    seqs, fk, fv = _plan(cl)
    nc = _build(seqs, fk, fv)
    ktb, vtb, qtb = _shard_inputs(q, kc, vc, bt, cl, seqs, fk, fv)

    in_maps = [{"kt": ktb[c], "vt": vtb[c], "qt": qtb} for c in range(NC)]
    kwargs = {}
    if _TRACE["on"]:
        kwargs = dict(trace=True, trace_cores=list(range(NC)))
    res = bass_utils.run_bass_kernel_spmd(
        nc, in_maps, core_ids=list(range(NC)), **kwargs
    )
    _TRACE["result"] = res

    o = np.zeros((D, S * H), np.float64)
    a = np.zeros((1, S * H), np.float64)
    for c in range(NC):
        o += res.results[c]["outp"].astype(np.float64)
        a += res.results[c]["acc"].astype(np.float64)
    a = a.reshape(S, H)
    for s, info in enumerate(seqs):
        a[s] -= NC * info["L"] - info["ctx"]  # remove exp(0)=1 pad junk
    out = o.reshape(D, S, H).transpose(1, 2, 0) / a[:, :, None]
    return out.astype(np.float32)


# revision 24
# speedup vs baseline: 1.0348x; 1.0262x over previous
"""PagedAttention decode kernel for 8 Trainium2 NeuronCores.

Problem (hardcoded shapes):
  query        [32, 16, 128]  f32
  key_cache    [4096, 16, 16, 16, 8]   f32  ([NB, H, D/x, Bsz, x])
  value_cache  [4096, 16, 128, 16]     f32  ([NB, H, D, Bsz])
  block_tables [32, 128] i32
  context_lens [32] i32
  out          [32, 16, 128] f32

Sharding strategy: context-parallel. Every core processes all 32 sequences,
holding every-8th KV block of each sequence (~1/8 of each context). Each core
computes unnormalized attention numerators (sum_t exp(l_t) * v_t) and softmax
partial sums; the host sums partials across cores and normalizes. exp() is
taken without max-subtraction (logits are O(5), safe in fp32), so partial
softmax combination needs no max rescaling.

Device layout per core (host-prepared, bf16):
  kt [128, FK]  per seq: [d=128, h=16, L_s] K^T, zero-padded to L_s
  vt [128, FV]  per seq, per 128-token chunk: [t_chunk, h=16, d=128]
  qt [128, 512] q * 1/sqrt(128), laid out [d, s, h]
Outputs (f32):
  outp [128, 512]  unnormalized out, [d, s, h]
  acc  [16, 32]    sum over tokens of exp(logit) incl. zero-pad junk
                   (pad tokens give logit exactly 0 -> exp 1; host subtracts
                   the known pad count)

Pipeline per (core, seq), chunked by 128 tokens (nch = ceil(L/128) <= 2):
  1. DMA K^T tile [128, 16*L]
  2. QK matmuls per (head, chunk): lhsT = K_h chunk [d, tc] stationary,
     rhs = q_h [d, 1] -> logits^T [t, h] in PSUM (tokens on partitions)
  3. one ACT exp per seq: PSUM [128, nch*16] -> SBUF bf16 (junk rows beyond
     partial chunks are harmless, never read)
  4. denominator: ones-vector matmul  sum_t p[t, h] -> [1, 16] PSUM
  5. V matmuls per (chunk, head): out[d, h] += V_chunk_h^T p_chunk_h
  6. DVE copies PSUM->SBUF; one final DMA for all seqs
"""

import math

import numpy as np
import ml_dtypes

SCALE = 0.08838834764831845  # 1/sqrt(128)
NC = 8
S = 32
H = 16
D = 128
BSZ = 16
BF16 = ml_dtypes.bfloat16

# test.py hook: set _TRACE["on"]=True before calling kernel() to profile.
_TRACE = {"on": False, "result": None}


def _plan(cl):
    # process longest sequences first: the pipeline drains on small tail work
    order = sorted(range(S), key=lambda s: -int(cl[s]))
    seqs = []
    fk = 0
    fv = 0
    rot = 0
    for sid in order:
        ctx = int(cl[sid])
        nb = -(-ctx // BSZ)          # blocks in use for this seq
        pcb = -(-nb // NC)           # blocks per core (ceil)
        L = pcb * BSZ                # per-core token count (16-aligned)
        nch = -(-L // 128)           # 128-token chunks (last may be partial)
        # partial chunks occupy few partitions -> few DMA engines; rotate
        # their partition base (PE tile_position allows 0/32/64 only for
        # small tiles) to spread descriptor load
        tcl = L - (L // 128) * 128
        if 0 < tcl <= 32:
            bases = (0, 32, 64)
        elif 0 < tcl <= 64:
            bases = (0, 64)
        else:
            bases = (0,)
        pbase = bases[rot % len(bases)]
        rot += 1
        seqs.append(dict(sid=sid, ctx=ctx, nb=nb, pcb=pcb, L=L, nch=nch,
                         koff=fk, voff=fv, pbase=pbase))
        fk += H * L
        fv += nch * H * D
    return seqs, fk, fv


def _build(seqs, fk, fv):
    import concourse.mybir as mybir
    import concourse.tile as tile
    from concourse import bacc

    nc = bacc.Bacc("TRN2", target_bir_lowering=False, debug=False, num_devices=NC)
    kt = nc.dram_tensor("kt", [D, fk], mybir.dt.bfloat16, kind="ExternalInput").ap()
    vt = nc.dram_tensor("vt", [D, fv], mybir.dt.bfloat16, kind="ExternalInput").ap()
    qt = nc.dram_tensor("qt", [D, S * H], mybir.dt.bfloat16, kind="ExternalInput").ap()
    outp = nc.dram_tensor("outp", [D, S * H], mybir.dt.float32, kind="ExternalOutput").ap()
    acc = nc.dram_tensor("acc", [1, S * H], mybir.dt.float32, kind="ExternalOutput").ap()

    with tile.TileContext(nc) as tc:
        with tc.tile_pool(name="cst", bufs=1) as cst, \
             tc.tile_pool(name="kp", bufs=8) as kp, \
             tc.tile_pool(name="vp", bufs=8) as vp, \
             tc.tile_pool(name="pp", bufs=S) as pp, \
             tc.tile_pool(name="lg", bufs=6, space="PSUM") as lg, \
             tc.tile_pool(name="po", bufs=1, space="PSUM") as po, \
             tc.tile_pool(name="dn", bufs=1, space="PSUM") as dn:
            qsb = cst.tile([D, S * H], mybir.dt.bfloat16, tag="q")
            nc.sync.dma_start(qsb[:, :], qt[:, :])
            ones = cst.tile([D, 1], mybir.dt.bfloat16, tag="ones")
            nc.vector.memset(ones[:, :], 1.0)
            osb = cst.tile([D, S * H], mybir.dt.float32, tag="o")
            asb = cst.tile([1, S * H], mybir.dt.float32, tag="a")
            # all 32 sequences share one PSUM bank each for out/denominator:
            # no slot rotation -> PE never stalls on PSUM slot releases
            pout = po.tile([D, S * H], mybir.dt.float32, tag="po")
            dnp = dn.tile([1, S * H], mybir.dt.float32, tag="dn")

            for s, info in enumerate(seqs):
                L = info["L"]
                nch = info["nch"]
                koff = info["koff"]
                voff = info["voff"]

                ktile = kp.tile([D, H * L], mybir.dt.bfloat16, tag="k")
                nc.sync.dma_start(ktile[:, :], kt[:, koff:koff + H * L])

                vtile = vp.tile([D, nch * H * D], mybir.dt.bfloat16, tag="v")
                nfull = L // 128
                tcl = L - nfull * 128
                pb = info["pbase"]
                if nfull:
                    nc.scalar.dma_start(
                        vtile[:, : nfull * H * D], vt[:, voff:voff + nfull * H * D]
                    )
                if tcl:
                    nc.scalar.dma_start(
                        vtile[pb:pb + tcl, nfull * H * D: nch * H * D],
                        vt[pb:pb + tcl, voff + nfull * H * D: voff + nch * H * D],
                    )

                def cbase(c):
                    return pb if (tcl and c == nch - 1) else 0

                # logits^T: [token, head] per chunk, tokens on partitions
                ptp = lg.tile([D, nch * H], mybir.dt.float32, tag="lg")
                for c in range(nch):
                    tc_c = min(128, L - c * 128)
                    b = cbase(c)
                    for h in range(H):
                        nc.tensor.matmul(
                            ptp[b:b + tc_c, c * H + h: c * H + h + 1],
                            ktile[:, h * L + c * 128: h * L + c * 128 + tc_c],
                            qsb[:, s * H + h:s * H + h + 1],
                            start=True,
                            stop=True,
                        )

                psb = pp.tile([D, nch * H], mybir.dt.bfloat16, tag="p")
                nc.scalar.activation(
                    psb[:, :], ptp[:, :], mybir.ActivationFunctionType.Exp
                )

                # one closed accumulation group per PSUM column: chunk loop
                # INSIDE the head loop (a start=True re-arms the whole 2KB
                # zero region, so open groups must not interleave per zone)
                for c in range(nch):
                    tc_c = min(128, L - c * 128)
                    b = cbase(c)
                    nc.tensor.matmul(
                        dnp[0:1, s * H:(s + 1) * H],
                        ones[b:b + tc_c, :],
                        psb[b:b + tc_c, c * H:(c + 1) * H],
                        start=(c == 0),
                        stop=(c == nch - 1),
                    )
                for h in range(H):
                    for c in range(nch):
                        tc_c = min(128, L - c * 128)
                        b = cbase(c)
                        nc.tensor.matmul(
                            pout[:, s * H + h:s * H + h + 1],
                            vtile[b:b + tc_c, c * H * D + h * D: c * H * D + (h + 1) * D],
                            psb[b:b + tc_c, c * H + h: c * H + h + 1],
                            start=(c == 0),
                            stop=(c == nch - 1),
                        )

            # ACT (not DVE) copies: keeps every matmul's cross-engine deps
            # on the single ACT semaphore — the MM ISA slot fits one wait.
            nc.scalar.copy(osb[:, :], pout[:, :])
            nc.scalar.copy(asb[:, :], dnp[:, :])
            nc.sync.dma_start(outp[:, :], osb[:, :])
            nc.sync.dma_start(acc[:, :], asb[:, :])
    nc.compile()
    return nc


def _shard_inputs(q, kc, vc, bt, cl, seqs, fk, fv):
    """Build per-core kt/vt and the shared qt."""
    kts = [np.zeros((D, fk), np.float32) for _ in range(NC)]
    vts = [np.zeros((D, fv), np.float32) for _ in range(NC)]

    for s, info in enumerate(seqs):
        ctx, nb, pcb, L = info["ctx"], info["nb"], info["pcb"], info["L"]
        koff, voff = info["koff"], info["voff"]
        idx = bt[info["sid"], :nb]
        kb = kc[idx]                       # [nb, H, 16, 16, 8] (copy)
        vb = vc[idx]                       # [nb, H, D, 16] (copy)
        rem = ctx - (nb - 1) * BSZ         # 1..16 tokens in last block
        if rem < BSZ:
            kb[-1, :, :, rem:, :] = 0
            vb[-1, :, :, rem:] = 0
        kbp = np.zeros((NC * pcb, H, BSZ, BSZ, 8), np.float32)
        kbp[:nb] = kb
        vbp = np.zeros((NC * pcb, H, D, BSZ), np.float32)
        vbp[:nb] = vb
        # core c owns blocks c, c+8, c+16, ... of this sequence
        kk = (
            kbp.reshape(pcb, NC, H, BSZ, BSZ, 8)
            .transpose(1, 3, 5, 2, 0, 4)
            .reshape(NC, D, H, pcb * BSZ)
        )
        vv = (
            vbp.reshape(pcb, NC, H, D, BSZ)
            .transpose(1, 0, 4, 2, 3)
            .reshape(NC, pcb * BSZ, H, D)
        )
        L0 = pcb * BSZ
        for c in range(NC):
            kpad = np.zeros((D, H, L), np.float32)
            kpad[:, :, :L0] = kk[c]
            kts[c][:, koff:koff + H * L] = kpad.reshape(D, H * L)
            vpad = np.zeros((L, H, D), np.float32)
            vpad[:L0] = vv[c]
            nfull = L // 128
            for ck in range(info["nch"]):
                tc_c = min(128, L - ck * 128)
                b = info["pbase"] if (ck == nfull and tc_c < 128) else 0
                slab = vpad[ck * 128: ck * 128 + tc_c].reshape(tc_c, H * D)
                vts[c][b:b + tc_c, voff + ck * H * D: voff + (ck + 1) * H * D] = slab

    order = [info["sid"] for info in seqs]
    qtb = (
        (q.astype(np.float32)[order] * SCALE)
        .transpose(2, 0, 1)
        .reshape(D, S * H)
        .astype(BF16)
    )
    ktb = [a.astype(BF16) for a in kts]
    vtb = [a.astype(BF16) for a in vts]
    return ktb, vtb, qtb


def kernel(query, key_cache, value_cache, block_tables, context_lens):
    from concourse import bass_utils

    q = np.asarray(query, dtype=np.float32)
    kc = np.asarray(key_cache, dtype=np.float32)
    vc = np.asarray(value_cache, dtype=np.float32)
    bt = np.asarray(block_tables).astype(np.int64)
    cl = np.asarray(context_lens).astype(np.int64)
    assert q.shape == (S, H, D) and bt.shape[0] == S and cl.shape == (S,)

    seqs, fk, fv = _plan(cl)
    nc = _build(seqs, fk, fv)
    ktb, vtb, qtb = _shard_inputs(q, kc, vc, bt, cl, seqs, fk, fv)

    in_maps = [{"kt": ktb[c], "vt": vtb[c], "qt": qtb} for c in range(NC)]
    kwargs = {}
    if _TRACE["on"]:
        kwargs = dict(trace=True, trace_cores=list(range(NC)))
    res = bass_utils.run_bass_kernel_spmd(
        nc, in_maps, core_ids=list(range(NC)), **kwargs
    )
    _TRACE["result"] = res

    o = np.zeros((D, S * H), np.float64)
    a = np.zeros((1, S * H), np.float64)
    for c in range(NC):
        o += res.results[c]["outp"].astype(np.float64)
        a += res.results[c]["acc"].astype(np.float64)
    a = a.reshape(S, H)
    for s, info in enumerate(seqs):
        a[s] -= NC * info["L"] - info["ctx"]  # remove exp(0)=1 pad junk
    out_slot = o.reshape(D, S, H).transpose(1, 2, 0) / a[:, :, None]
    out = np.empty_like(out_slot)
    out[[info["sid"] for info in seqs]] = out_slot
    return out.astype(np.float32)
